# revision 1
# baseline (speedup 1.0000x reference)
"""BLSTM-LM Trainium2 kernel.

Model: B=4, T=512, V=32000, E=512, H=512 (fp32 reference).
  e = emb[x]; fwd/bwd LSTM over T; out = concat(h_f, h_b) @ proj_w.T + proj_b

Two SPMD launches:
  A) recurrence: core 0 = forward LSTM, core 1 = backward LSTM (as a forward
     pass over the time-reversed embedding sequence). Weight-stationary PE
     tiling producing the gate pre-activations directly in a transposed
     [128, 64] layout (partition = gate-dim chunk, free = (k,b)), so the
     ACT/DVE gate math runs on all 128 partitions. Cell state kept fp32;
     matmuls in bf16 (fp32 matmul is 4 cyc/row on TRN2 PE, bf16 is 1).
  B) projection: all 8 cores, vocab-sharded (4000 cols each), bf16 matmul
     with proj bias folded in as a 9th contraction tile (ones row x bias row).
"""

import os
import sys

sys.path.insert(0, "/opt/trn_rl_repo")
# No NTFF/axon profile hook in this container; a stray BASS_TRACE=1 would
# crash run_bass_kernel_spmd (ModuleNotFoundError: antenv.axon_hooks).
os.environ["BASS_NEVER_TRACE"] = "1"

import ml_dtypes
import numpy as np

import concourse.bass as bass
import concourse.tile as tile
from concourse import bacc, mybir
from concourse.bass_utils import run_bass_kernel_spmd

BF16 = mybir.dt.bfloat16
F8 = mybir.dt.float8e4
F32 = mybir.dt.float32
f8np = ml_dtypes.float8_e4m3
AF = mybir.ActivationFunctionType
bf16 = ml_dtypes.bfloat16

B, T, V, E, H = 4, 512, 32000, 512, 512
G = 4 * H  # 2048 gate rows, order i|f|o|u
NCORES = 8
VS = V // NCORES  # 4000 vocab cols per core
KE = E // 128  # 4 contraction tiles over E
KH = H // 128  # 4 contraction tiles over H
MG = G // 128  # 16 output tiles over gate rows
# m-tile emission order: i(0-3), f(4-7), u(12-15) first -> psA; o(8-11) last -> psB
M_IFU = [0, 1, 2, 3, 4, 5, 6, 7, 12, 13, 14, 15]
M_O = [8, 9, 10, 11]


def emit_recurrence(nc, t_len, eT, wihT, whhT, bihT, seq):
    NB = t_len * B
    with tile.TileContext(nc) as tc:
        with (
            tc.tile_pool(name="wp", bufs=1) as wp,
            tc.tile_pool(name="big", bufs=1) as big,
            tc.tile_pool(name="st", bufs=3) as st,
            tc.tile_pool(name="wk", bufs=3) as wk,
            tc.tile_pool(name="pIF", bufs=2, space=bass.MemorySpace.PSUM) as pIF,
            tc.tile_pool(name="pU", bufs=2, space=bass.MemorySpace.PSUM) as pU,
            tc.tile_pool(name="pO", bufs=2, space=bass.MemorySpace.PSUM) as pO,
            tc.tile_pool(name="pG", bufs=2, space=bass.MemorySpace.PSUM) as pG,
        ):
            # --- weights / inputs to SBUF ---
            eS = wp.tile([128, KE * NB], BF16)
            wS = wp.tile([128, KE * G], BF16)
            hS = wp.tile([128, KH * G], F8)  # fp8 recurrent weights: FWL loads 4/cyc
            bS = wp.tile([128, MG], F32)
            for k in range(KE):
                nc.sync.dma_start(eS[:, k * NB : (k + 1) * NB], eT[k * 128 : (k + 1) * 128, :])
                nc.sync.dma_start(wS[:, k * G : (k + 1) * G], wihT[k * 128 : (k + 1) * 128, :])
            for k in range(KH):
                nc.sync.dma_start(hS[:, k * G : (k + 1) * G], whhT[k * 128 : (k + 1) * 128, :])
            nc.sync.dma_start(bS[:], bihT[:, :])

            gx = big.tile([128, t_len * 64], BF16)  # [p, t*64 + gate4*... ] = m*4+b within step
            sq = big.tile([128, t_len * 16], BF16)  # h history, [p, t*16 + k*4 + b]
            gx3 = gx[:].rearrange("p (t q) -> p t q", q=64)

            # --- gx = e @ w_ih.T + b_ih, written transposed+interleaved ---
            CH = 512
            nch = max(1, NB // CH)
            cw = min(CH, NB)
            for m in range(MG):
                for n in range(nch):
                    ps = pG.tile([128, cw], F32)
                    for k in range(KE):
                        nc.tensor.matmul(
                            ps[:, :],
                            wS[:, k * G + m * 128 : k * G + (m + 1) * 128],
                            eS[:, k * NB + n * cw : k * NB + (n + 1) * cw],
                            start=(k == 0),
                            stop=(k == KE - 1),
                        )
                    t0, t1 = (n * cw) // 4, ((n + 1) * cw) // 4
                    dst = gx3[:, t0:t1, m * 4 : (m + 1) * 4]
                    src = ps[:].rearrange("p (t b) -> p t b", b=4)
                    nc.scalar.activation(dst, src, AF.Identity, bias=bS[:, m : m + 1])

            # --- zero initial state ---
            h0 = st.tile([128, 16], F8, tag="h0")
            c0 = st.tile([128, 16], F32, tag="c0")
            nc.vector.memset(h0[:], 0.0)
            nc.vector.memset(c0[:], 0.0)

            c_prev = c0[:]
            h_prev = h0[:]
            for t in range(t_len):
                hprev = h_prev
                pu = pU.tile([128, 16], F32)
                pif = pIF.tile([128, 32], F32)
                po = pO.tile([128, 16], F32)

                def mm_group(m, out):
                    for k in range(KH):
                        nc.tensor.matmul(
                            out,
                            hS[:, k * G + m * 128 : k * G + (m + 1) * 128],
                            hprev[:, k * 4 : (k + 1) * 4],
                            start=(k == 0),
                            stop=(k == KH - 1),
                        )

                gxs = gx[:, t * 64 : (t + 1) * 64]
                # u first: tanh(u) overlaps the i/f/o matmuls
                for m in (12, 13, 14, 15):
                    mm_group(m, pu[:, (m - 12) * 4 : (m - 11) * 4])
                gu = wk.tile([128, 16], F32, tag="gu")
                nc.vector.tensor_add(gu[:], pu[:], gxs[:, 48:64])
                tu = wk.tile([128, 16], F32, tag="tu")
                nc.scalar.activation(tu[:], gu[:], AF.Tanh)
                # i, f next: sigmoid + c-chain overlap the o matmuls
                for m in (0, 1, 2, 3, 4, 5, 6, 7):
                    mm_group(m, pif[:, m * 4 : (m + 1) * 4])
                gif = wk.tile([128, 32], F32, tag="gif")
                nc.vector.tensor_add(gif[:], pif[:], gxs[:, 0:32])
                sif = wk.tile([128, 32], F32, tag="sif")
                nc.scalar.activation(sif[:], gif[:], AF.Sigmoid)
                iu = wk.tile([128, 16], F32, tag="iu")
                fc = wk.tile([128, 16], F32, tag="fc")
                nc.vector.tensor_mul(iu[:], sif[:, 0:16], tu[:])
                nc.vector.tensor_mul(fc[:], sif[:, 16:32], c_prev)
                c_new = st.tile([128, 16], F32, tag="c")
                nc.vector.tensor_add(c_new[:], fc[:], iu[:])
                tc_ = wk.tile([128, 16], F32, tag="tc")
                nc.scalar.activation(tc_[:], c_new[:], AF.Tanh)
                # o last: its short add+sigmoid tail runs after the final MMs
                for m in (8, 9, 10, 11):
                    mm_group(m, po[:, (m - 8) * 4 : (m - 7) * 4])
                go = wk.tile([128, 16], F32, tag="go")
                nc.vector.tensor_add(go[:], po[:], gxs[:, 32:48])
                so = wk.tile([128, 16], F32, tag="so")
                nc.scalar.activation(so[:], go[:], AF.Sigmoid)
                # fp8 copy feeds the next step's matmul (critical path);
                # full-precision bf16 h goes to the sequence buffer (off-path)
                h8 = st.tile([128, 16], F8, tag="h8")
                nc.vector.tensor_mul(h8[:], so[:], tc_[:])
                nc.vector.tensor_mul(sq[:, t * 16 : (t + 1) * 16], so[:], tc_[:])
                c_prev = c_new[:]
                h_prev = h8[:]

            nc.sync.dma_start(seq[:, :], sq[:])
    return nc


def build_recurrence(t_len=T):
    nc = bacc.Bacc(None, target_bir_lowering=False)
    NB = t_len * B
    eT = nc.dram_tensor("eT", [E, NB], BF16, kind="ExternalInput")
    wihT = nc.dram_tensor("wihT", [E, G], BF16, kind="ExternalInput")
    whhT = nc.dram_tensor("whhT", [H, G], F8, kind="ExternalInput")
    bihT = nc.dram_tensor("bihT", [128, MG], F32, kind="ExternalInput")
    seq = nc.dram_tensor("seq", [128, t_len * 16], BF16, kind="ExternalOutput")
    emit_recurrence(nc, t_len, eT[:], wihT[:], whhT[:], bihT[:], seq[:])
    nc.finalize()
    return nc


def emit_projection(nc, hcT, pwT, out):
    NB = T * B  # 2048
    K9 = 9  # 8 tiles of hcat (2H=1024) + 1 bias tile
    NCH = 8
    CW = VS // NCH  # 500
    with tile.TileContext(nc) as tc:
        with (
            tc.tile_pool(name="wp", bufs=1) as wp,
            tc.tile_pool(name="ob", bufs=6) as ob,
            tc.tile_pool(name="pp", bufs=4, space=bass.MemorySpace.PSUM) as pp,
        ):
            hc = wp.tile([128, K9 * NB], BF16)
            pw = wp.tile([128, K9 * VS], BF16)
            for k in range(K9):
                nc.sync.dma_start(hc[:, k * NB : (k + 1) * NB], hcT[k * 128 : (k + 1) * 128, :])
                nc.sync.dma_start(pw[:, k * VS : (k + 1) * VS], pwT[k * 128 : (k + 1) * 128, :])
            for n in range(NCH):
                for m in range(NB // 128):
                    ps = pp.tile([128, CW], F32)
                    for k in range(K9):
                        nc.tensor.matmul(
                            ps[:, :],
                            hc[:, k * NB + m * 128 : k * NB + (m + 1) * 128],
                            pw[:, k * VS + n * CW : k * VS + (n + 1) * CW],
                            start=(k == 0),
                            stop=(k == K9 - 1),
                        )
                    o = ob.tile([128, CW], F32, tag="o")
                    if (n * 16 + m) % 2 == 0:
                        nc.vector.tensor_copy(o[:], ps[:])
                    else:
                        nc.scalar.activation(o[:], ps[:], AF.Copy)
                    nc.sync.dma_start(
                        out[m * 128 : (m + 1) * 128, n * CW : (n + 1) * CW], o[:]
                    )
    return nc


def build_projection():
    nc = bacc.Bacc(None, target_bir_lowering=False)
    NB = T * B
    hcT = nc.dram_tensor("hcT", [9 * 128, NB], BF16, kind="ExternalInput")
    pwT = nc.dram_tensor("pwT", [9 * 128, VS], BF16, kind="ExternalInput")
    out = nc.dram_tensor("out", [NB, VS], F32, kind="ExternalOutput")
    emit_projection(nc, hcT[:], pwT[:], out[:])
    nc.finalize()
    return nc


_NC_CACHE = {}
LAST_TIMES = {}


def _get_nc(name):
    if name not in _NC_CACHE:
        _NC_CACHE[name] = build_recurrence() if name == "rec" else build_projection()
    return _NC_CACHE[name]


def _prep_dir(e_bte, w_ih, b_ih, w_hh):
    """Per-direction host prep. e_bte: [B,T,E] fp32 (already time-ordered for
    this direction). Returns the in_map for one recurrence core."""
    eT = np.ascontiguousarray(e_bte.transpose(2, 1, 0).reshape(E, T * B)).astype(bf16)
    wihT = np.ascontiguousarray(w_ih.T).astype(bf16)
    whhT = np.ascontiguousarray(w_hh.T).astype(f8np)
    bihT = np.ascontiguousarray(b_ih.reshape(MG, 128).T).astype(np.float32)
    return {"eT": eT, "wihT": wihT, "whhT": whhT, "bihT": bihT}


def _seq_from_raw(raw):
    """raw [128, T*16] bf16 -> seqT [H, T*B] fp32: h[k*128+p, t*4+b] = raw[p, t*16+k*4+b]."""
    r = np.asarray(raw).reshape(128, T, 4, 4)
    return np.ascontiguousarray(r.transpose(2, 0, 1, 3).reshape(H, T * B))


def kernel(x, emb, w_ih_f, b_ih_f, w_hh_f, w_ih_b, b_ih_b, w_hh_b, proj_w, proj_b):
    x = np.asarray(x)
    e = np.asarray(emb)[x]  # [B,T,E] fp32 gather (host: input staging)
    m_f = _prep_dir(e, np.asarray(w_ih_f), np.asarray(b_ih_f), np.asarray(w_hh_f))
    m_b = _prep_dir(
        e[:, ::-1, :], np.asarray(w_ih_b), np.asarray(b_ih_b), np.asarray(w_hh_b)
    )

    import time as _time

    nc_rec = _get_nc("rec")
    _t = _time.perf_counter()
    res = run_bass_kernel_spmd(nc_rec, [m_f, m_b], [0, 1]).results
    LAST_TIMES["rec"] = _time.perf_counter() - _t
    seq_f = _seq_from_raw(res[0]["seq"])  # [512, 2048]
    seq_b_rev = _seq_from_raw(res[1]["seq"])
    # bwd ran forward over reversed time; un-reverse the t axis
    seq_b = np.ascontiguousarray(
        seq_b_rev.reshape(H, T, B)[:, ::-1, :].reshape(H, T * B)
    )

    hcT = np.zeros((9 * 128, T * B), np.float32)
    hcT[0:512] = seq_f
    hcT[512:1024] = seq_b
    hcT[1024] = 1.0
    hcT = hcT.astype(bf16)

    pw = np.asarray(proj_w)
    pb = np.asarray(proj_b)
    maps = []
    for c in range(NCORES):
        pwT = np.zeros((9 * 128, VS), np.float32)
        pwT[0:1024] = pw[c * VS : (c + 1) * VS, :].T
        pwT[1024] = pb[c * VS : (c + 1) * VS]
        maps.append({"hcT": hcT, "pwT": pwT.astype(bf16)})

    nc_proj = _get_nc("proj")
    _t = _time.perf_counter()
    res2 = run_bass_kernel_spmd(nc_proj, maps, list(range(NCORES))).results
    LAST_TIMES["proj"] = _time.perf_counter() - _t
    out = np.concatenate([np.asarray(r["out"], np.float32) for r in res2], axis=1)
    # out rows are t*4+b
    return np.ascontiguousarray(
        out.reshape(T, B, V).transpose(1, 0, 2)
    ).astype(np.float32)



# revision 2
# speedup vs baseline: 1.1082x; 1.1082x over previous
"""BLSTM-LM Trainium2 kernel, v2: single SPMD launch, dynamic loops.

Model: B=4, T=512, V=32000, E=512, H=512 (fp32 reference).
  e = emb[x]; fwd/bwd LSTM over T; out = concat(h_f, h_b) @ proj_w.T + proj_b

One SPMD launch on all 8 cores. Every core runs BOTH directions'
recurrences (redundant across cores, ~2ms) and then its own vocab slice
(V/8 = 4000 columns) of the output projection. This trades a little
redundant device compute for: one compile instead of two, one PJRT
dispatch, and no host roundtrip between recurrence and projection.

The T=512 recurrence runs as a Tile dynamic For_i loop (body = one
timestep, ~76 instructions) instead of being fully unrolled: the BIR
program drops from ~39k instructions to ~600, which is what dominates
wall time (walrus compile is ~4ms/instruction).

Layouts (per direction):
  eT   [E, T*B]   bf16, col = t*4+b (bwd direction gets time-reversed e)
  gx   [128, T*64] bf16 in SBUF: gx[p, t*64 + m*4 + b], gate row = m*128+p
  h/c state [128, 16]: state[p, k*4+b], h row = k*128+p
  sq   [128, T*16] bf16: h history, sq[p, t*16 + k*4 + b]
Projection consumes sq_f/sq_b directly as matmul weights via strided 3D
APs (no reshuffle): lhsT(m,k) = sq[:, (m*32:+32)*16 + k*4 (+4)].
"""

import os
import sys

sys.path.insert(0, "/opt/trn_rl_repo")
os.environ["BASS_NEVER_TRACE"] = "1"

import ml_dtypes
import numpy as np

import concourse.bass as bass
import concourse.tile as tile
from concourse import bacc, mybir
from concourse.bass import ds
from concourse.bass_utils import run_bass_kernel_spmd

BF16 = mybir.dt.bfloat16
F8 = mybir.dt.float8e4
F32 = mybir.dt.float32
f8np = ml_dtypes.float8_e4m3
AF = mybir.ActivationFunctionType
bf16 = ml_dtypes.bfloat16

B, T, V, E, H = 4, 512, 32000, 512, 512
G = 4 * H  # 2048 gate rows, order i|f|o|u
NB = T * B  # 2048
NCORES = 8
VS = V // NCORES  # 4000 vocab cols per core
KE = E // 128  # 4 contraction tiles over E
KH = H // 128  # 4 contraction tiles over H
MG = G // 128  # 16 gate row tiles
MNB = NB // 128  # 16 output row tiles
NCH = 8  # vocab chunks per core
CW = VS // NCH  # 500 cols per chunk


def emit_dir(nc, tc, dram, bufs, d):
    """Emit gx compute + recurrence for one direction d ('f'/'b').

    The bwd direction runs forward over the time-reversed embedding
    sequence; its h history is written time-reversed (dst offset
    (T-1-t)*16) so sq_b ends up in original time order and the
    projection can read sq_f/sq_b identically."""
    eS, wS, hS, bS, gx, gps, st, wk, pU, pIF, pO = bufs
    eT, wihT, whhT, bihT = dram[f"eT{d}"], dram[f"wihT{d}"], dram[f"whhT{d}"], dram[f"bihT{d}"]
    for k in range(KE):
        nc.sync.dma_start(eS[:, k * NB : (k + 1) * NB], eT[k * 128 : (k + 1) * 128, :])
        nc.sync.dma_start(wS[:, k * G : (k + 1) * G], wihT[k * 128 : (k + 1) * 128, :])
    for k in range(KH):
        nc.sync.dma_start(hS[:, k * G : (k + 1) * G], whhT[k * 128 : (k + 1) * 128, :])
    nc.sync.dma_start(bS[:], bihT[:, :])

    gx3 = gx[:].rearrange("p (t q) -> p t q", q=64)

    # gx = e @ w_ih.T + b_ih, transposed+interleaved: dynamic loop over
    # 4 column chunks of 512 (= 128 timesteps each).
    with tc.For_i(0, 4, 1) as n:
        for m in range(MG):
            ps = gps.tile([128, 512], F32, tag="gps")
            for k in range(KE):
                nc.tensor.matmul(
                    ps[:, :],
                    wS[:, k * G + m * 128 : k * G + (m + 1) * 128],
                    eS[:, ds(n * 512 + k * NB, 512)],
                    start=(k == 0),
                    stop=(k == KE - 1),
                )
            dst = gx3[:, ds(n * 128, 128), m * 4 : (m + 1) * 4]
            src = ps[:].rearrange("p (t b) -> p t b", b=4)
            nc.scalar.activation(dst, src, AF.Identity, bias=bS[:, m : m + 1])

    # recurrence: one timestep per For_i iteration
    h0 = st.tile([128, 16], F8, tag="h0")
    c0 = st.tile([128, 16], F32, tag="c0")
    sq = st.tile([128, T * 16], BF16, tag=f"sq{d}")
    nc.vector.memset(h0[:], 0.0)
    nc.vector.memset(c0[:], 0.0)

    with tc.For_i(0, T, 1) as t:
        pu = pU.tile([128, 16], F32, tag="pu")
        pif = pIF.tile([128, 32], F32, tag="pif")
        po = pO.tile([128, 16], F32, tag="po")

        def mm_group(m, out):
            for k in range(KH):
                nc.tensor.matmul(
                    out,
                    hS[:, k * G + m * 128 : k * G + (m + 1) * 128],
                    h0[:, k * 4 : (k + 1) * 4],
                    start=(k == 0),
                    stop=(k == KH - 1),
                )

        # u first: tanh(u) overlaps the i/f/o matmuls
        for m in (12, 13, 14, 15):
            mm_group(m, pu[:, (m - 12) * 4 : (m - 11) * 4])
        gu = wk.tile([128, 16], F32, tag="gu")
        nc.vector.tensor_add(gu[:], pu[:], gx[:, ds(t * 64 + 48, 16)])
        tu = wk.tile([128, 16], F32, tag="tu")
        nc.scalar.activation(tu[:], gu[:], AF.Tanh)
        # i, f next
        for m in (0, 1, 2, 3, 4, 5, 6, 7):
            mm_group(m, pif[:, m * 4 : (m + 1) * 4])
        gif = wk.tile([128, 32], F32, tag="gif")
        nc.vector.tensor_add(gif[:], pif[:], gx[:, ds(t * 64, 32)])
        sif = wk.tile([128, 32], F32, tag="sif")
        nc.scalar.activation(sif[:], gif[:], AF.Sigmoid)
        iu = wk.tile([128, 16], F32, tag="iu")
        fc = wk.tile([128, 16], F32, tag="fc")
        nc.vector.tensor_mul(iu[:], sif[:, 0:16], tu[:])
        nc.vector.tensor_mul(fc[:], sif[:, 16:32], c0[:])
        # c0 <- fc + iu (inputs don't include c0; Tile orders the WAR)
        nc.vector.tensor_add(c0[:], fc[:], iu[:])
        tc_ = wk.tile([128, 16], F32, tag="tc")
        nc.scalar.activation(tc_[:], c0[:], AF.Tanh)
        # o last
        for m in (8, 9, 10, 11):
            mm_group(m, po[:, (m - 8) * 4 : (m - 7) * 4])
        go = wk.tile([128, 16], F32, tag="go")
        nc.vector.tensor_add(go[:], po[:], gx[:, ds(t * 64 + 32, 16)])
        so = wk.tile([128, 16], F32, tag="so")
        nc.scalar.activation(so[:], go[:], AF.Sigmoid)
        nc.vector.tensor_mul(h0[:], so[:], tc_[:])
        if d == "f":
            nc.vector.tensor_mul(sq[:, ds(t * 16, 16)], so[:], tc_[:])
        else:
            nc.vector.tensor_mul(sq[:, ds((T - 1) * 16 - t * 16, 16)], so[:], tc_[:])
    return sq


def emit_kernel(nc):
    dram = {}
    for d in ("f", "b"):
        dram[f"eT{d}"] = nc.dram_tensor(f"eT{d}", [E, NB], BF16, kind="ExternalInput")
        dram[f"wihT{d}"] = nc.dram_tensor(f"wihT{d}", [E, G], BF16, kind="ExternalInput")
        dram[f"whhT{d}"] = nc.dram_tensor(f"whhT{d}", [H, G], F8, kind="ExternalInput")
        dram[f"bihT{d}"] = nc.dram_tensor(f"bihT{d}", [128, MG], F32, kind="ExternalInput")
    pwT = nc.dram_tensor("pwT", [8 * 128, VS], BF16, kind="ExternalInput")
    pbR = nc.dram_tensor("pbR", [1, VS], BF16, kind="ExternalInput")
    out = nc.dram_tensor("out", [NB, VS], F32, kind="ExternalOutput")
    # out rows b-major: out[b*T + t, v]
    outR = out[:].rearrange("(b t) v -> t b v", b=B)

    with tile.TileContext(nc) as tc:
        with (
            tc.tile_pool(name="wp", bufs=1) as wp,
            tc.tile_pool(name="st", bufs=1) as st,
            tc.tile_pool(name="wk", bufs=2) as wk,
            tc.tile_pool(name="pw", bufs=2) as pwp,
            tc.tile_pool(name="ob", bufs=4) as ob,
            tc.tile_pool(name="gps", bufs=2, space=bass.MemorySpace.PSUM) as gps,
            tc.tile_pool(name="pU", bufs=1, space=bass.MemorySpace.PSUM) as pU,
            tc.tile_pool(name="pIF", bufs=1, space=bass.MemorySpace.PSUM) as pIF,
            tc.tile_pool(name="pO", bufs=1, space=bass.MemorySpace.PSUM) as pO,
            tc.tile_pool(name="pp", bufs=2, space=bass.MemorySpace.PSUM) as pp,
        ):
            eS = wp.tile([128, KE * NB], BF16)
            wS = wp.tile([128, KE * G], BF16)
            hS = wp.tile([128, KH * G], F8)
            bS = wp.tile([128, MG], F32)
            gx = wp.tile([128, T * 64], BF16)
            # bias tile for projection: row 0 = pb slice, rows 1.. = 0
            pbS = wp.tile([128, VS], BF16)
            onesT = wp.tile([128, 128], BF16)
            nc.vector.memset(pbS[:], 0.0)
            nc.vector.memset(onesT[:], 0.0)
            nc.vector.memset(onesT[0:1, :], 1.0)
            nc.sync.dma_start(pbS[0:1, :], pbR[:, :])

            bufs = (eS, wS, hS, bS, gx, gps, st, wk, pU, pIF, pO)
            sq_f = emit_dir(nc, tc, dram, bufs, "f")
            sq_b = emit_dir(nc, tc, dram, bufs, "b")
            sq3_f = sq_f[:].rearrange("p (t q) -> p t q", q=16)
            sq3_b = sq_b[:].rearrange("p (t q) -> p t q", q=16)

            # reshuffle h history into contiguous matmul-weight layout:
            # hcS[p, k*2048 + t*4 + b] = h_k[k*128+p] at (t, b).
            # Matmul weights can't take 2-free-dim strided APs, so this
            # materializes them; reuses gx's SBUF slot (dead after rec b).
            hcS = wp.tile([128, 8 * NB], BF16, tag="gx")
            for k in range(8):
                sq3 = sq3_f if k < 4 else sq3_b
                kk = k % 4
                nc.vector.tensor_copy(
                    hcS[:, k * NB : (k + 1) * NB].rearrange("p (t b) -> p t b", b=B),
                    sq3[:, :, kk * 4 : (kk + 1) * 4],
                )

            # projection: out[nb, v] = sum_h hcat[h, nb] pw[v, h] + pb[v]
            # loop over 8 vocab chunks of 500; weights streamed from HBM.
            with tc.For_i(0, NCH, 1) as n:
                pwS = pwp.tile([128, 8 * CW], BF16, tag="pwS")
                for k in range(8):
                    nc.sync.dma_start(
                        pwS[:, k * CW : (k + 1) * CW],
                        pwT[k * 128 : (k + 1) * 128, ds(n * CW, CW)],
                    )
                for m in range(MNB):
                    ps = pp.tile([128, CW], F32, tag="pps")
                    for k in range(8):
                        nc.tensor.matmul(
                            ps[:, :],
                            hcS[:, k * NB + m * 128 : k * NB + (m + 1) * 128],
                            pwS[:, k * CW : (k + 1) * CW],
                            start=(k == 0),
                            stop=False,
                        )
                    nc.tensor.matmul(
                        ps[:, :],
                        onesT[:, :],
                        pbS[:, ds(n * CW, CW)],
                        start=False,
                        stop=True,
                    )
                    o = ob.tile([128, CW], F32, tag="o")
                    if m % 2 == 0:
                        nc.vector.tensor_copy(o[:], ps[:])
                    else:
                        nc.scalar.activation(o[:], ps[:], AF.Copy)
                    nc.sync.dma_start(
                        outR[m * 32 : (m + 1) * 32, :, ds(n * CW, CW)], o[:]
                    )
    return nc


def build():
    nc = bacc.Bacc(None, target_bir_lowering=False)
    emit_kernel(nc)
    nc.finalize()
    return nc


_NC_CACHE = {}
LAST_TIMES = {}


def _get_nc():
    if "k" not in _NC_CACHE:
        _NC_CACHE["k"] = build()
    return _NC_CACHE["k"]


def _prep_dir(e_bte, w_ih, b_ih, w_hh, d):
    eT = np.ascontiguousarray(e_bte.transpose(2, 1, 0).reshape(E, T * B)).astype(bf16)
    wihT = np.ascontiguousarray(w_ih.T).astype(bf16)
    whhT = np.ascontiguousarray(w_hh.T).astype(f8np)
    bihT = np.ascontiguousarray(b_ih.reshape(MG, 128).T).astype(np.float32)
    return {f"eT{d}": eT, f"wihT{d}": wihT, f"whhT{d}": whhT, f"bihT{d}": bihT}


def prep_maps(x, emb, w_ih_f, b_ih_f, w_hh_f, w_ih_b, b_ih_b, w_hh_b, proj_w, proj_b):
    x = np.asarray(x)
    e = np.asarray(emb)[x]  # [B,T,E] host gather
    base = {}
    base.update(_prep_dir(e, np.asarray(w_ih_f), np.asarray(b_ih_f), np.asarray(w_hh_f), "f"))
    base.update(
        _prep_dir(e[:, ::-1, :], np.asarray(w_ih_b), np.asarray(b_ih_b), np.asarray(w_hh_b), "b")
    )
    pw = np.asarray(proj_w).astype(bf16)
    pb = np.asarray(proj_b).astype(bf16)

    maps = []
    for c in range(NCORES):
        m = dict(base)
        m["pwT"] = np.ascontiguousarray(pw[c * VS : (c + 1) * VS, :].T)
        m["pbR"] = np.ascontiguousarray(pb[c * VS : (c + 1) * VS].reshape(1, VS))
        maps.append(m)
    return maps


def kernel(x, emb, w_ih_f, b_ih_f, w_hh_f, w_ih_b, b_ih_b, w_hh_b, proj_w, proj_b):
    import time as _time

    maps = prep_maps(
        x, emb, w_ih_f, b_ih_f, w_hh_f, w_ih_b, b_ih_b, w_hh_b, proj_w, proj_b
    )
    nc = _get_nc()
    _t = _time.perf_counter()
    res = run_bass_kernel_spmd(nc, maps, list(range(NCORES))).results
    LAST_TIMES["launch"] = _time.perf_counter() - _t

    full = np.concatenate([np.asarray(r["out"], np.float32) for r in res], axis=1)
    return full.reshape(B, T, V)


# revision 3
# speedup vs baseline: 4.0159x; 3.6237x over previous
"""BLSTM-LM Trainium2 kernel, v2: single SPMD launch, dynamic loops.

Model: B=4, T=512, V=32000, E=512, H=512 (fp32 reference).
  e = emb[x]; fwd/bwd LSTM over T; out = concat(h_f, h_b) @ proj_w.T + proj_b

One SPMD launch on all 8 cores. Every core runs BOTH directions'
recurrences (redundant across cores, ~2ms) and then its own vocab slice
(V/8 = 4000 columns) of the output projection. This trades a little
redundant device compute for: one compile instead of two, one PJRT
dispatch, and no host roundtrip between recurrence and projection.

The T=512 recurrence runs as a Tile dynamic For_i loop (body = one
timestep) instead of being fully unrolled: the BIR program drops from
~39k instructions to ~1.7k, which collapses compile/serialization time
(the old unrolled kernel spent ~150s there).

Precision: fp16 activations/weights (not bf16 — same PE speed, 8x finer
mantissa; all magnitudes here are <<1e4 so no overflow risk), fp8e4m3
recurrent weights (PE fast-weight-load, 4 rows/cycle), fp32 PSUM
accumulation, fp16 output (halves the 262MB result fetch; adds ~2e-4
abs error on values <=0.45).

Layouts (per direction):
  eT   [E, T*B]   f16, col = t*4+b (shared by both directions; the bwd
                  pass reads gx with time-reversed dynamic offsets)
  gx   [128, T*64] f16 in SBUF: gx[p, t*64 + m*4 + b], gate row = m*128+p
  h/c state [128, 16]: state[p, k*4+b], h row = k*128+p
  sq   [128, T*16] f16: h history in original time order for both dirs
  hcS  [128, 8*T*B] f16: h history reshuffled to matmul-weight layout
"""

import os
import sys

sys.path.insert(0, "/opt/trn_rl_repo")
os.environ["BASS_NEVER_TRACE"] = "1"

import ml_dtypes
import numpy as np

import concourse.bass as bass
import concourse.tile as tile
from concourse import bacc, mybir
from concourse.bass import ds
from concourse.bass_utils import run_bass_kernel_spmd

F16 = mybir.dt.float16
F8 = mybir.dt.float8e4
F32 = mybir.dt.float32
f8np = ml_dtypes.float8_e4m3
AF = mybir.ActivationFunctionType

B, T, V, E, H = 4, 512, 32000, 512, 512
G = 4 * H  # 2048 gate rows, order i|f|o|u
NB = T * B  # 2048
NCORES = 8
VS = V // NCORES  # 4000 vocab cols per core
KE = E // 128  # 4 contraction tiles over E
KH = H // 128  # 4 contraction tiles over H
MG = G // 128  # 16 gate row tiles
MNB = NB // 128  # 16 output row tiles
NCH = 8  # vocab chunks per core
CW = VS // NCH  # 500 cols per chunk


def emit_dir(nc, tc, dram, bufs, d):
    """Emit gx compute + recurrence for one direction d ('f'/'b').

    Both directions share eS (the embedding sequence in original time
    order). The bwd pass runs its recurrence loop backwards through gx
    via reversed dynamic offsets, and its h history is written at the
    original-time position, so sq_b ends up in original time order."""
    eS, wS, hS, bS, gx, gps, st, wk, pU, pIF, pO = bufs
    wihT, whhT, bihT = dram[f"wihT{d}"], dram[f"whhT{d}"], dram[f"bihT{d}"]
    if d == "f":
        eT = dram["eT"]
        for k in range(KE):
            nc.sync.dma_start(eS[:, k * NB : (k + 1) * NB], eT[k * 128 : (k + 1) * 128, :])
    for k in range(KE):
        nc.sync.dma_start(wS[:, k * G : (k + 1) * G], wihT[k * 128 : (k + 1) * 128, :])
    for k in range(KH):
        nc.sync.dma_start(hS[:, k * G : (k + 1) * G], whhT[k * 128 : (k + 1) * 128, :])
    nc.sync.dma_start(bS[:], bihT[:, :])

    gx3 = gx[:].rearrange("p (t q) -> p t q", q=64)

    # gx = e @ w_ih.T + b_ih, transposed+interleaved: dynamic loop over
    # 4 column chunks of 512 (= 128 timesteps each).
    with tc.For_i(0, 4, 1) as n:
        for m in range(MG):
            ps = gps.tile([128, 512], F32, tag="gps")
            for k in range(KE):
                nc.tensor.matmul(
                    ps[:, :],
                    wS[:, k * G + m * 128 : k * G + (m + 1) * 128],
                    eS[:, ds(n * 512 + k * NB, 512)],
                    start=(k == 0),
                    stop=(k == KE - 1),
                )
            dst = gx3[:, ds(n * 128, 128), m * 4 : (m + 1) * 4]
            src = ps[:].rearrange("p (t b) -> p t b", b=4)
            nc.scalar.activation(dst, src, AF.Identity, bias=bS[:, m : m + 1])

    # recurrence: one timestep per For_i iteration. Loop step t reads
    # original time tau = t (fwd) or T-1-t (bwd); h lands at sq[tau].
    h0 = st.tile([128, 16], F16, tag="h0")
    c0 = st.tile([128, 16], F32, tag="c0")
    sq = st.tile([128, T * 16], F16, tag=f"sq{d}")
    nc.vector.memset(h0[:], 0.0)
    nc.vector.memset(c0[:], 0.0)

    with tc.For_i(0, T, 1) as t:
        tau64 = t * 64 if d == "f" else (T - 1) * 64 - t * 64
        tau16 = t * 16 if d == "f" else (T - 1) * 16 - t * 16
        pu = pU.tile([128, 16], F32, tag="pu")
        pif = pIF.tile([128, 32], F32, tag="pif")
        po = pO.tile([128, 16], F32, tag="po")

        def mm_group(m, out):
            for k in range(KH):
                nc.tensor.matmul(
                    out,
                    hS[:, k * G + m * 128 : k * G + (m + 1) * 128],
                    h0[:, k * 4 : (k + 1) * 4],
                    start=(k == 0),
                    stop=(k == KH - 1),
                )

        # u first: tanh(u) overlaps the i/f/o matmuls
        for m in (12, 13, 14, 15):
            mm_group(m, pu[:, (m - 12) * 4 : (m - 11) * 4])
        gu = wk.tile([128, 16], F32, tag="gu")
        nc.vector.tensor_add(gu[:], pu[:], gx[:, ds(tau64 + 48, 16)])
        tu = wk.tile([128, 16], F32, tag="tu")
        nc.scalar.activation(tu[:], gu[:], AF.Tanh)
        # i, f next
        for m in (0, 1, 2, 3, 4, 5, 6, 7):
            mm_group(m, pif[:, m * 4 : (m + 1) * 4])
        gif = wk.tile([128, 32], F32, tag="gif")
        nc.vector.tensor_add(gif[:], pif[:], gx[:, ds(tau64, 32)])
        sif = wk.tile([128, 32], F32, tag="sif")
        nc.scalar.activation(sif[:], gif[:], AF.Sigmoid)
        iu = wk.tile([128, 16], F32, tag="iu")
        fc = wk.tile([128, 16], F32, tag="fc")
        nc.vector.tensor_mul(iu[:], sif[:, 0:16], tu[:])
        nc.vector.tensor_mul(fc[:], sif[:, 16:32], c0[:])
        # c0 <- fc + iu (inputs don't include c0; Tile orders the WAR)
        nc.vector.tensor_add(c0[:], fc[:], iu[:])
        tc_ = wk.tile([128, 16], F32, tag="tc")
        nc.scalar.activation(tc_[:], c0[:], AF.Tanh)
        # o last
        for m in (8, 9, 10, 11):
            mm_group(m, po[:, (m - 8) * 4 : (m - 7) * 4])
        go = wk.tile([128, 16], F32, tag="go")
        nc.vector.tensor_add(go[:], po[:], gx[:, ds(tau64 + 32, 16)])
        so = wk.tile([128, 16], F32, tag="so")
        nc.scalar.activation(so[:], go[:], AF.Sigmoid)
        nc.vector.tensor_mul(h0[:], so[:], tc_[:])
        nc.vector.tensor_mul(sq[:, ds(tau16, 16)], so[:], tc_[:])
    return sq


def emit_kernel(nc):
    dram = {"eT": nc.dram_tensor("eT", [E, NB], F16, kind="ExternalInput")}
    for d in ("f", "b"):
        dram[f"wihT{d}"] = nc.dram_tensor(f"wihT{d}", [E, G], F16, kind="ExternalInput")
        dram[f"whhT{d}"] = nc.dram_tensor(f"whhT{d}", [H, G], F8, kind="ExternalInput")
        dram[f"bihT{d}"] = nc.dram_tensor(f"bihT{d}", [128, MG], F32, kind="ExternalInput")
    pwT = nc.dram_tensor("pwT", [8 * 128, VS], F16, kind="ExternalInput")
    pbR = nc.dram_tensor("pbR", [1, VS], F16, kind="ExternalInput")
    out = nc.dram_tensor("out", [NB, VS], F16, kind="ExternalOutput")
    # out rows b-major: out[b*T + t, v]
    outR = out[:].rearrange("(b t) v -> t b v", b=B)

    with tile.TileContext(nc) as tc:
        with (
            tc.tile_pool(name="wp", bufs=1) as wp,
            tc.tile_pool(name="st", bufs=1) as st,
            tc.tile_pool(name="wk", bufs=2) as wk,
            tc.tile_pool(name="pw", bufs=2) as pwp,
            tc.tile_pool(name="ob", bufs=4) as ob,
            tc.tile_pool(name="gps", bufs=2, space=bass.MemorySpace.PSUM) as gps,
            tc.tile_pool(name="pU", bufs=1, space=bass.MemorySpace.PSUM) as pU,
            tc.tile_pool(name="pIF", bufs=1, space=bass.MemorySpace.PSUM) as pIF,
            tc.tile_pool(name="pO", bufs=1, space=bass.MemorySpace.PSUM) as pO,
            tc.tile_pool(name="pp", bufs=2, space=bass.MemorySpace.PSUM) as pp,
        ):
            eS = wp.tile([128, KE * NB], F16)
            wS = wp.tile([128, KE * G], F16)
            hS = wp.tile([128, KH * G], F8)
            bS = wp.tile([128, MG], F32)
            gx = wp.tile([128, T * 64], F16)
            # bias tile for projection: row 0 = pb slice, rows 1.. = 0
            pbS = wp.tile([128, VS], F16)
            onesT = wp.tile([128, 128], F16)
            nc.vector.memset(pbS[:], 0.0)
            nc.vector.memset(onesT[:], 0.0)
            nc.vector.memset(onesT[0:1, :], 1.0)
            nc.sync.dma_start(pbS[0:1, :], pbR[:, :])

            bufs = (eS, wS, hS, bS, gx, gps, st, wk, pU, pIF, pO)
            sq_f = emit_dir(nc, tc, dram, bufs, "f")
            sq_b = emit_dir(nc, tc, dram, bufs, "b")
            sq3_f = sq_f[:].rearrange("p (t q) -> p t q", q=16)
            sq3_b = sq_b[:].rearrange("p (t q) -> p t q", q=16)

            # reshuffle h history into contiguous matmul-weight layout:
            # hcS[p, k*2048 + t*4 + b] = h_k[k*128+p] at (t, b).
            # Matmul weights can't take 2-free-dim strided APs, so this
            # materializes them; reuses gx's SBUF slot (dead after rec b).
            hcS = wp.tile([128, 8 * NB], F16, tag="gx")
            for k in range(8):
                sq3 = sq3_f if k < 4 else sq3_b
                kk = k % 4
                nc.vector.tensor_copy(
                    hcS[:, k * NB : (k + 1) * NB].rearrange("p (t b) -> p t b", b=B),
                    sq3[:, :, kk * 4 : (kk + 1) * 4],
                )

            # projection: out[nb, v] = sum_h hcat[h, nb] pw[v, h] + pb[v]
            # loop over 8 vocab chunks of 500; weights streamed from HBM.
            with tc.For_i(0, NCH, 1) as n:
                pwS = pwp.tile([128, 8 * CW], F16, tag="pwS")
                for k in range(8):
                    nc.sync.dma_start(
                        pwS[:, k * CW : (k + 1) * CW],
                        pwT[k * 128 : (k + 1) * 128, ds(n * CW, CW)],
                    )
                for m in range(MNB):
                    ps = pp.tile([128, CW], F32, tag="pps")
                    for k in range(8):
                        nc.tensor.matmul(
                            ps[:, :],
                            hcS[:, k * NB + m * 128 : k * NB + (m + 1) * 128],
                            pwS[:, k * CW : (k + 1) * CW],
                            start=(k == 0),
                            stop=False,
                        )
                    nc.tensor.matmul(
                        ps[:, :],
                        onesT[:, :],
                        pbS[:, ds(n * CW, CW)],
                        start=False,
                        stop=True,
                    )
                    o = ob.tile([128, CW], F16, tag="o")
                    if m % 2 == 0:
                        nc.vector.tensor_copy(o[:], ps[:])
                    else:
                        nc.scalar.activation(o[:], ps[:], AF.Copy)
                    nc.sync.dma_start(
                        outR[m * 32 : (m + 1) * 32, :, ds(n * CW, CW)], o[:]
                    )
    return nc


def build():
    nc = bacc.Bacc(None, target_bir_lowering=False)
    emit_kernel(nc)
    nc.finalize()
    return nc


_NC_CACHE = {}
LAST_TIMES = {}


def _get_nc():
    if "k" not in _NC_CACHE:
        _NC_CACHE["k"] = build()
    return _NC_CACHE["k"]


def prep_maps(x, emb, w_ih_f, b_ih_f, w_hh_f, w_ih_b, b_ih_b, w_hh_b, proj_w, proj_b):
    x = np.asarray(x)
    e = np.asarray(emb)[x]  # [B,T,E] host gather
    base = {
        "eT": np.ascontiguousarray(e.transpose(2, 1, 0).reshape(E, T * B)).astype(
            np.float16
        )
    }
    for d, w_ih, b_ih, w_hh in (
        ("f", w_ih_f, b_ih_f, w_hh_f),
        ("b", w_ih_b, b_ih_b, w_hh_b),
    ):
        base[f"wihT{d}"] = np.ascontiguousarray(np.asarray(w_ih).T).astype(np.float16)
        base[f"whhT{d}"] = np.ascontiguousarray(np.asarray(w_hh).T).astype(f8np)
        base[f"bihT{d}"] = np.ascontiguousarray(
            np.asarray(b_ih).reshape(MG, 128).T
        ).astype(np.float32)
    pw = np.asarray(proj_w).astype(np.float16)
    pb = np.asarray(proj_b).astype(np.float16)

    maps = []
    for c in range(NCORES):
        m = dict(base)
        m["pwT"] = np.ascontiguousarray(pw[c * VS : (c + 1) * VS, :].T)
        m["pbR"] = np.ascontiguousarray(pb[c * VS : (c + 1) * VS].reshape(1, VS))
        maps.append(m)
    return maps


def kernel(x, emb, w_ih_f, b_ih_f, w_hh_f, w_ih_b, b_ih_b, w_hh_b, proj_w, proj_b):
    import time as _time

    maps = prep_maps(
        x, emb, w_ih_f, b_ih_f, w_hh_f, w_ih_b, b_ih_b, w_hh_b, proj_w, proj_b
    )
    nc = _get_nc()
    _t = _time.perf_counter()
    res = run_bass_kernel_spmd(nc, maps, list(range(NCORES))).results
    LAST_TIMES["launch"] = _time.perf_counter() - _t

    full = np.concatenate([np.asarray(r["out"]) for r in res], axis=1)
    return full.reshape(B, T, V).astype(np.float32)


# revision 4
# speedup vs baseline: 22.7196x; 5.6574x over previous
"""BLSTM-LM Trainium2 kernel, v2: single SPMD launch, dynamic loops.

Model: B=4, T=512, V=32000, E=512, H=512 (fp32 reference).
  e = emb[x]; fwd/bwd LSTM over T; out = concat(h_f, h_b) @ proj_w.T + proj_b

One SPMD launch on all 8 cores. Every core runs BOTH directions'
recurrences (redundant across cores, ~2ms) and then its own vocab slice
(V/8 = 4000 columns) of the output projection. This trades a little
redundant device compute for: one compile instead of two, one PJRT
dispatch, and no host roundtrip between recurrence and projection.

The T=512 recurrence runs as a Tile dynamic For_i loop (body = one
timestep) instead of being fully unrolled: the BIR program drops from
~39k instructions to ~1.7k, which collapses compile/serialization time
(the old unrolled kernel spent ~150s there).

Precision: fp16 activations/weights (not bf16 — same PE speed, 8x finer
mantissa; all magnitudes here are <<1e4 so no overflow risk), fp8e4m3
recurrent weights (PE fast-weight-load, 4 rows/cycle), fp32 PSUM
accumulation, fp16 output (halves the 262MB result fetch; adds ~2e-4
abs error on values <=0.45).

Layouts (per direction):
  eT   [E, T*B]   f16, col = t*4+b (shared by both directions; the bwd
                  pass reads gx with time-reversed dynamic offsets)
  gx   [128, T*64] f16 in SBUF: gx[p, t*64 + m*4 + b], gate row = m*128+p
  h/c state [128, 16]: state[p, k*4+b], h row = k*128+p
  sq   [128, T*16] f16: h history in original time order for both dirs
  hcS  [128, 8*T*B] f16: h history reshuffled to matmul-weight layout
"""

import os
import sys

sys.path.insert(0, "/opt/trn_rl_repo")
os.environ["BASS_NEVER_TRACE"] = "1"

import ml_dtypes
import numpy as np

import concourse.bass as bass
import concourse.tile as tile
from concourse import bacc, mybir
from concourse.bass import ds

F16 = mybir.dt.float16
F8 = mybir.dt.float8e4
F32 = mybir.dt.float32
f8np = ml_dtypes.float8_e4m3
AF = mybir.ActivationFunctionType

B, T, V, E, H = 4, 512, 32000, 512, 512
G = 4 * H  # 2048 gate rows, order i|f|o|u
NB = T * B  # 2048
NCORES = 8
VS = V // NCORES  # 4000 vocab cols per core
KE = E // 128  # 4 contraction tiles over E
KH = H // 128  # 4 contraction tiles over H
MG = G // 128  # 16 gate row tiles
MNB = NB // 128  # 16 output row tiles
NCH = 8  # vocab chunks per core
CW = VS // NCH  # 500 cols per chunk


def emit_dir(nc, tc, dram, bufs, d):
    """Emit gx compute + recurrence for one direction d ('f'/'b').

    Both directions share eS (the embedding sequence in original time
    order). The bwd pass runs its recurrence loop backwards through gx
    via reversed dynamic offsets, and its h history is written at the
    original-time position, so sq_b ends up in original time order."""
    eS, wS, hS, bS, gx, gps, st, wk, pU, pIF, pO = bufs
    wihT, whhT, bihT = dram[f"wihT{d}"], dram[f"whhT{d}"], dram[f"bihT{d}"]
    if d == "f":
        eT = dram["eT"]
        for k in range(KE):
            nc.sync.dma_start(eS[:, k * NB : (k + 1) * NB], eT[k * 128 : (k + 1) * 128, :])
    for k in range(KE):
        nc.sync.dma_start(wS[:, k * G : (k + 1) * G], wihT[k * 128 : (k + 1) * 128, :])
    for k in range(KH):
        nc.sync.dma_start(hS[:, k * G : (k + 1) * G], whhT[k * 128 : (k + 1) * 128, :])
    nc.sync.dma_start(bS[:], bihT[:, :])

    gx3 = gx[:].rearrange("p (t q) -> p t q", q=64)

    # gx = e @ w_ih.T + b_ih, transposed+interleaved: dynamic loop over
    # 4 column chunks of 512 (= 128 timesteps each).
    with tc.For_i(0, 4, 1) as n:
        for m in range(MG):
            ps = gps.tile([128, 512], F32, tag="gps")
            for k in range(KE):
                nc.tensor.matmul(
                    ps[:, :],
                    wS[:, k * G + m * 128 : k * G + (m + 1) * 128],
                    eS[:, ds(n * 512 + k * NB, 512)],
                    start=(k == 0),
                    stop=(k == KE - 1),
                )
            dst = gx3[:, ds(n * 128, 128), m * 4 : (m + 1) * 4]
            src = ps[:].rearrange("p (t b) -> p t b", b=4)
            nc.scalar.activation(dst, src, AF.Identity, bias=bS[:, m : m + 1])

    # recurrence: one timestep per For_i iteration. Loop step t reads
    # original time tau = t (fwd) or T-1-t (bwd); h lands at sq[tau].
    h0 = st.tile([128, 16], F16, tag="h0")
    c0 = st.tile([128, 16], F32, tag="c0")
    sq = st.tile([128, T * 16], F16, tag=f"sq{d}")
    nc.vector.memset(h0[:], 0.0)
    nc.vector.memset(c0[:], 0.0)

    with tc.For_i(0, T, 1) as t:
        tau64 = t * 64 if d == "f" else (T - 1) * 64 - t * 64
        tau16 = t * 16 if d == "f" else (T - 1) * 16 - t * 16
        pu = pU.tile([128, 16], F32, tag="pu")
        pif = pIF.tile([128, 32], F32, tag="pif")
        po = pO.tile([128, 16], F32, tag="po")

        def mm_group(m, out):
            for k in range(KH):
                nc.tensor.matmul(
                    out,
                    hS[:, k * G + m * 128 : k * G + (m + 1) * 128],
                    h0[:, k * 4 : (k + 1) * 4],
                    start=(k == 0),
                    stop=(k == KH - 1),
                )

        # u first: tanh(u) overlaps the i/f/o matmuls
        for m in (12, 13, 14, 15):
            mm_group(m, pu[:, (m - 12) * 4 : (m - 11) * 4])
        gu = wk.tile([128, 16], F32, tag="gu")
        nc.vector.tensor_add(gu[:], pu[:], gx[:, ds(tau64 + 48, 16)])
        tu = wk.tile([128, 16], F32, tag="tu")
        nc.scalar.activation(tu[:], gu[:], AF.Tanh)
        # i, f next
        for m in (0, 1, 2, 3, 4, 5, 6, 7):
            mm_group(m, pif[:, m * 4 : (m + 1) * 4])
        gif = wk.tile([128, 32], F32, tag="gif")
        nc.vector.tensor_add(gif[:], pif[:], gx[:, ds(tau64, 32)])
        sif = wk.tile([128, 32], F32, tag="sif")
        nc.scalar.activation(sif[:], gif[:], AF.Sigmoid)
        iu = wk.tile([128, 16], F32, tag="iu")
        fc = wk.tile([128, 16], F32, tag="fc")
        nc.vector.tensor_mul(iu[:], sif[:, 0:16], tu[:])
        nc.vector.tensor_mul(fc[:], sif[:, 16:32], c0[:])
        # c0 <- fc + iu (inputs don't include c0; Tile orders the WAR)
        nc.vector.tensor_add(c0[:], fc[:], iu[:])
        tc_ = wk.tile([128, 16], F32, tag="tc")
        nc.scalar.activation(tc_[:], c0[:], AF.Tanh)
        # o last
        for m in (8, 9, 10, 11):
            mm_group(m, po[:, (m - 8) * 4 : (m - 7) * 4])
        go = wk.tile([128, 16], F32, tag="go")
        nc.vector.tensor_add(go[:], po[:], gx[:, ds(tau64 + 32, 16)])
        so = wk.tile([128, 16], F32, tag="so")
        nc.scalar.activation(so[:], go[:], AF.Sigmoid)
        nc.vector.tensor_mul(h0[:], so[:], tc_[:])
        nc.vector.tensor_mul(sq[:, ds(tau16, 16)], so[:], tc_[:])
    return sq


def emit_kernel(nc):
    dram = {"eT": nc.dram_tensor("eT", [E, NB], F16, kind="ExternalInput")}
    for d in ("f", "b"):
        dram[f"wihT{d}"] = nc.dram_tensor(f"wihT{d}", [E, G], F16, kind="ExternalInput")
        dram[f"whhT{d}"] = nc.dram_tensor(f"whhT{d}", [H, G], F8, kind="ExternalInput")
        dram[f"bihT{d}"] = nc.dram_tensor(f"bihT{d}", [128, MG], F32, kind="ExternalInput")
    pwT = nc.dram_tensor("pwT", [8 * 128, VS], F16, kind="ExternalInput")
    pbR = nc.dram_tensor("pbR", [1, VS], F16, kind="ExternalInput")
    out = nc.dram_tensor("out", [NB, VS], F16, kind="ExternalOutput")
    # out rows b-major: out[b*T + t, v]
    outR = out[:].rearrange("(b t) v -> t b v", b=B)

    with tile.TileContext(nc) as tc:
        with (
            tc.tile_pool(name="wp", bufs=1) as wp,
            tc.tile_pool(name="st", bufs=1) as st,
            tc.tile_pool(name="wk", bufs=2) as wk,
            tc.tile_pool(name="pw", bufs=2) as pwp,
            tc.tile_pool(name="ob", bufs=4) as ob,
            tc.tile_pool(name="gps", bufs=2, space=bass.MemorySpace.PSUM) as gps,
            tc.tile_pool(name="pU", bufs=1, space=bass.MemorySpace.PSUM) as pU,
            tc.tile_pool(name="pIF", bufs=1, space=bass.MemorySpace.PSUM) as pIF,
            tc.tile_pool(name="pO", bufs=1, space=bass.MemorySpace.PSUM) as pO,
            tc.tile_pool(name="pp", bufs=2, space=bass.MemorySpace.PSUM) as pp,
        ):
            eS = wp.tile([128, KE * NB], F16)
            wS = wp.tile([128, KE * G], F16)
            hS = wp.tile([128, KH * G], F8)
            bS = wp.tile([128, MG], F32)
            gx = wp.tile([128, T * 64], F16)
            # bias tile for projection: row 0 = pb slice, rows 1.. = 0
            pbS = wp.tile([128, VS], F16)
            onesT = wp.tile([128, 128], F16)
            nc.vector.memset(pbS[:], 0.0)
            nc.vector.memset(onesT[:], 0.0)
            nc.vector.memset(onesT[0:1, :], 1.0)
            nc.sync.dma_start(pbS[0:1, :], pbR[:, :])

            bufs = (eS, wS, hS, bS, gx, gps, st, wk, pU, pIF, pO)
            sq_f = emit_dir(nc, tc, dram, bufs, "f")
            sq_b = emit_dir(nc, tc, dram, bufs, "b")
            sq3_f = sq_f[:].rearrange("p (t q) -> p t q", q=16)
            sq3_b = sq_b[:].rearrange("p (t q) -> p t q", q=16)

            # reshuffle h history into contiguous matmul-weight layout:
            # hcS[p, k*2048 + t*4 + b] = h_k[k*128+p] at (t, b).
            # Matmul weights can't take 2-free-dim strided APs, so this
            # materializes them; reuses gx's SBUF slot (dead after rec b).
            hcS = wp.tile([128, 8 * NB], F16, tag="gx")
            for k in range(8):
                sq3 = sq3_f if k < 4 else sq3_b
                kk = k % 4
                nc.vector.tensor_copy(
                    hcS[:, k * NB : (k + 1) * NB].rearrange("p (t b) -> p t b", b=B),
                    sq3[:, :, kk * 4 : (kk + 1) * 4],
                )

            # projection: out[nb, v] = sum_h hcat[h, nb] pw[v, h] + pb[v]
            # loop over 8 vocab chunks of 500; weights streamed from HBM.
            with tc.For_i(0, NCH, 1) as n:
                pwS = pwp.tile([128, 8 * CW], F16, tag="pwS")
                for k in range(8):
                    nc.sync.dma_start(
                        pwS[:, k * CW : (k + 1) * CW],
                        pwT[k * 128 : (k + 1) * 128, ds(n * CW, CW)],
                    )
                for m in range(MNB):
                    ps = pp.tile([128, CW], F32, tag="pps")
                    for k in range(8):
                        nc.tensor.matmul(
                            ps[:, :],
                            hcS[:, k * NB + m * 128 : k * NB + (m + 1) * 128],
                            pwS[:, k * CW : (k + 1) * CW],
                            start=(k == 0),
                            stop=False,
                        )
                    nc.tensor.matmul(
                        ps[:, :],
                        onesT[:, :],
                        pbS[:, ds(n * CW, CW)],
                        start=False,
                        stop=True,
                    )
                    o = ob.tile([128, CW], F16, tag="o")
                    if m % 2 == 0:
                        nc.vector.tensor_copy(o[:], ps[:])
                    else:
                        nc.scalar.activation(o[:], ps[:], AF.Copy)
                    nc.sync.dma_start(
                        outR[m * 32 : (m + 1) * 32, :, ds(n * CW, CW)], o[:]
                    )
    return nc


def build():
    nc = bacc.Bacc(None, target_bir_lowering=False)
    emit_kernel(nc)
    nc.finalize()
    return nc


_NC_CACHE = {}
LAST_TIMES = {}
PHASE_TIMES = {}


def _get_nc():
    if "k" not in _NC_CACHE:
        _NC_CACHE["k"] = build()
    return _NC_CACHE["k"]


def prep_maps(x, emb, w_ih_f, b_ih_f, w_hh_f, w_ih_b, b_ih_b, w_hh_b, proj_w, proj_b):
    x = np.asarray(x)
    e = np.asarray(emb)[x]  # [B,T,E] host gather
    base = {
        "eT": np.ascontiguousarray(e.transpose(2, 1, 0).reshape(E, T * B)).astype(
            np.float16
        )
    }
    for d, w_ih, b_ih, w_hh in (
        ("f", w_ih_f, b_ih_f, w_hh_f),
        ("b", w_ih_b, b_ih_b, w_hh_b),
    ):
        base[f"wihT{d}"] = np.ascontiguousarray(np.asarray(w_ih).T).astype(np.float16)
        base[f"whhT{d}"] = np.ascontiguousarray(np.asarray(w_hh).T).astype(f8np)
        base[f"bihT{d}"] = np.ascontiguousarray(
            np.asarray(b_ih).reshape(MG, 128).T
        ).astype(np.float32)
    pw = np.asarray(proj_w).astype(np.float16)
    pb = np.asarray(proj_b).astype(np.float16)

    maps = []
    for c in range(NCORES):
        m = dict(base)
        m["pwT"] = np.ascontiguousarray(pw[c * VS : (c + 1) * VS, :].T)
        m["pbR"] = np.ascontiguousarray(pb[c * VS : (c + 1) * VS].reshape(1, VS))
        maps.append(m)
    return maps


# Inputs that are identical on every core ride as replicated shard_map
# operands (one upload instead of eight).
_REPLICATED = {"eT", "wihTf", "wihTb", "whhTf", "whhTb", "bihTf", "bihTb"}


def _run(nc, maps):
    """Phase-timed replica of bass2jax.run_bass_via_pjrt with mixed
    replicated/sharded input specs. Returns the global 'out' array
    [NCORES*NB, VS] (vocab slice c in rows [c*NB:(c+1)*NB])."""
    import time as _time

    import jax
    from jax.sharding import Mesh, PartitionSpec
    from jax.experimental.shard_map import shard_map

    from concourse.bass2jax import (
        _bass_exec_p,
        install_neuronx_cc_hook,
        partition_id_tensor,
    )

    t0 = _time.perf_counter()
    install_neuronx_cc_hook()
    partition_name = nc.partition_id_tensor.name if nc.partition_id_tensor else None
    in_names, out_names, out_avals, zero_outs = [], [], [], []
    for alloc in nc.m.functions[0].allocations:
        if not isinstance(alloc, mybir.MemoryLocationSet):
            continue
        name = alloc.memorylocations[0].name
        if alloc.kind == "ExternalInput":
            if name != partition_name:
                in_names.append(name)
        elif alloc.kind == "ExternalOutput":
            out_names.append(name)
            shape = tuple(alloc.tensor_shape)
            dtype = mybir.dt.np(alloc.dtype)
            out_avals.append(jax.core.ShapedArray(shape, dtype))
            zero_outs.append(np.zeros((NCORES * shape[0], *shape[1:]), dtype))
    n_params = len(in_names)
    all_names = list(in_names) + list(out_names)
    if partition_name is not None:
        all_names.append(partition_name)

    def _body(*args):
        operands = list(args)
        if partition_name is not None:
            operands.append(partition_id_tensor())
        outs = _bass_exec_p.bind(
            *operands,
            out_avals=tuple(out_avals),
            in_names=tuple(all_names),
            out_names=tuple(out_names),
            lowering_input_output_aliases=(),
            sim_require_finite=True,
            sim_require_nnan=True,
            nc=nc,
        )
        return tuple(outs)

    devices = jax.devices()[:NCORES]
    mesh = Mesh(np.asarray(devices), ("core",))
    in_specs = tuple(
        PartitionSpec() if name in _REPLICATED else PartitionSpec("core")
        for name in in_names
    ) + (PartitionSpec("core"),) * len(out_names)
    out_specs = (PartitionSpec("core"),) * len(out_names)
    donate = tuple(range(n_params, n_params + len(out_names)))
    jitted = jax.jit(
        shard_map(
            _body, mesh=mesh, in_specs=in_specs, out_specs=out_specs, check_rep=False
        ),
        donate_argnums=donate,
        keep_unused=True,
    )

    args = []
    for name in in_names:
        if name in _REPLICATED:
            args.append(np.asarray(maps[0][name]))
        else:
            args.append(
                np.concatenate([np.asarray(m[name]) for m in maps], axis=0)
            )
    PHASE_TIMES["prep"] = _time.perf_counter() - t0

    t = _time.perf_counter()
    compiled = jitted.lower(*args, *zero_outs).compile()
    PHASE_TIMES["compile"] = _time.perf_counter() - t

    t = _time.perf_counter()
    out = compiled(*args, *zero_outs)
    jax.block_until_ready(out)
    PHASE_TIMES["upload_exec"] = _time.perf_counter() - t

    t = _time.perf_counter()
    res = np.asarray(out[0])
    PHASE_TIMES["fetch"] = _time.perf_counter() - t
    return res


def kernel(x, emb, w_ih_f, b_ih_f, w_hh_f, w_ih_b, b_ih_b, w_hh_b, proj_w, proj_b):
    import time as _time

    maps = prep_maps(
        x, emb, w_ih_f, b_ih_f, w_hh_f, w_ih_b, b_ih_b, w_hh_b, proj_w, proj_b
    )
    nc = _get_nc()
    _t = _time.perf_counter()
    glob = _run(nc, maps)  # [NCORES*NB, VS]
    LAST_TIMES["launch"] = _time.perf_counter() - _t

    full = np.concatenate(
        [glob[c * NB : (c + 1) * NB] for c in range(NCORES)], axis=1
    )
    return full.reshape(B, T, V).astype(np.float32)
